# revision 1
# baseline (speedup 1.0000x reference)
"""Trainium2 Bass kernel for the FNO-SMM problem (nn_FNO_SMM_34488587387600).

Data-parallel over 8 NeuronCores: 2 batches per core. Each core:
  - builds the nonuniform Fourier basis V (working set of 299 of the 552 rows;
    the rest are algebraically folded via conjugate symmetry) in two layouts
    (V^T [n,m] fp16 for the forward NUDFT, V [m,n] fp16 for the inverse),
    using host-range-reduced per-k angle tables + PE selection matmuls +
    DVE range-wrap + ACT sin
  - runs 4 spectral layers: forward NUDFT (col-tiled fp16 matmuls),
    per-mode channel mixing (augmented 2-mode block-diagonal fp16 matmuls),
    conjugate-folded inverse NUDFT fused with the 1x1 conv, exact-erf gelu
  - applies the fc1/fc2 head.

Host side does only input marshaling: global min/max, per-k angle tables
reduced to [-pi, pi] (sin LUT domain), weight repacking, and final gather
(+ fc2 bias scalar).
"""
import sys
import os

sys.path.insert(0, '/opt/trn_rl_repo')

import numpy as np
from contextlib import ExitStack

import concourse.bass as bass
import concourse.tile as tile
from concourse import bacc, mybir
from concourse.bass_utils import run_bass_kernel_spmd

MODES = 12
C = 32
N = 4096
B = 16
NCORES = 8
BL = B // NCORES          # 2 batches per core
NW = 299                  # working-set rows: 288 + 11 unpaired (kx=-12, ky<0)
NWP = 304                 # padded
PI = float(np.pi)

F32 = mybir.dt.float32
F32R = mybir.dt.float32r
F16 = mybir.dt.float16
AF = mybir.ActivationFunctionType
ALU = mybir.AluOpType

TRACE = False             # test harness can set kernel.TRACE = True
DEBUG = False             # adds intermediate-dump outputs

_CACHE = {}


# --------------------------------------------------------------------------
# host-side index helpers (python ints only; used at build/marshal time)
# --------------------------------------------------------------------------
def _w_rows():
    """W-set V-row indices: m in [0,288) then the 11 unpaired rows."""
    return list(range(288)) + [24 * j + 12 for j in range(12, 23)]


def _cap(t_ap, row0, nrows, pairs, free_off):
    """Custom AP on a tile's underlying tensor: rows [row0, row0+nrows),
    free pattern `pairs` ([[step, count], ...]) at element offset free_off."""
    base = t_ap.ap
    pstep = base[0][0]
    return bass.AP(tensor=t_ap.tensor, offset=row0 * pstep + free_off + t_ap.offset,
                   ap=[[pstep, nrows]] + [list(p) for p in pairs])


# --------------------------------------------------------------------------
# device program
# --------------------------------------------------------------------------
def _build_program():
    nc = bacc.Bacc("TRN2", target_bir_lowering=False, debug=False,
                   num_devices=NCORES)

    din = {}
    def dram_in(name, shape, dt):
        din[name] = nc.dram_tensor(name, list(shape), dt, kind="ExternalInput").ap()
        return din[name]

    ck_d = dram_in('ck', [BL, 49, N], F32R)
    selT_d = dram_in('selT', [49, 608], F32R)
    selInv_d = dram_in('selInv', [6, 49, 128], F32R)
    fc0w_d = dram_in('fc0w', [2, C], F32R)
    fc0b_d = dram_in('fc0b', [C, 1], F32)
    mmw_d = dram_in('mmw', [4, 144, 128, 128], F16)
    cwt_d = dram_in('cwt', [4, C, C], F16)
    cb_d = dram_in('cb', [4, C, 1], F32)
    fc1w_d = dram_in('fc1w', [C, 128], F16)
    fc1b_d = dram_in('fc1b', [128, 1], F32)
    fc2w_d = dram_in('fc2w', [128, 1], F16)
    i64_d = dram_in('i64', [C, C], F16)     # I * (1/64)
    is32_d = dram_in('is32', [C, C], F32)   # I * (1/32)
    js32_d = dram_in('js32', [C, C], F32)   # J * (1/32)

    y_d = nc.dram_tensor('y', [BL, N], F32, kind="ExternalOutput").ap()
    dbg = {}
    if DEBUG:
        dbg['h0'] = nc.dram_tensor('dbg_h0', [BL, C, N], F16, kind="ExternalOutput").ap()
        dbg['vt0'] = nc.dram_tensor('dbg_vt0', [128, 608], F16, kind="ExternalOutput").ap()
        dbg['vi0'] = nc.dram_tensor('dbg_vi0', [128, N], F16, kind="ExternalOutput").ap()
        dbg['R'] = nc.dram_tensor('dbg_R', [128, 288], F16, kind="ExternalOutput").ap()
        dbg['frs'] = nc.dram_tensor('dbg_frs', [C, NWP], F32, kind="ExternalOutput").ap()
        dbg['frx'] = nc.dram_tensor('dbg_frx', [C, NWP], F32, kind="ExternalOutput").ap()
        dbg['AT0'] = nc.dram_tensor('dbg_AT0', [128, C], F16, kind="ExternalOutput").ap()
        dbg['h1'] = nc.dram_tensor('dbg_h1', [BL, C, N], F16, kind="ExternalOutput").ap()

    with tile.TileContext(nc) as tc, ExitStack() as ctx:
        # ------------- persistent pool -------------
        pp = ctx.enter_context(tc.tile_pool(name="persist", bufs=1))
        vt = [[pp.tile([128, 608], F16, tag=f"vt{b}_{c}", name=f"vt{b}_{c}")
               for c in range(32)] for b in range(BL)]
        vrows = [128, 128, 48, 128, 128, 48]
        vinv = [[pp.tile([vrows[k], N], F16, tag=f"vi{b}_{k}", name=f"vi{b}_{k}")
                 for k in range(6)] for b in range(BL)]
        h = [pp.tile([C, N], F16, tag=f"h{b}", name=f"h{b}") for b in range(BL)]

        fc0w_t = pp.tile([2, C], F32R, tag="fc0w", name="fc0w_t")
        fc0b_t = pp.tile([C, 1], F32, tag="fc0b", name="fc0b_t")
        cwt_t = [pp.tile([C, C], F16, tag=f"cwt{l}", name=f"cwt{l}") for l in range(4)]
        cb_t = [pp.tile([C, 1], F32, tag=f"cb{l}", name=f"cb{l}") for l in range(4)]
        fc1w_t = pp.tile([C, 128], F16, tag="fc1w", name="fc1w_t")
        fc1b_t = pp.tile([128, 1], F32, tag="fc1b", name="fc1b_t")
        fc2w_t = pp.tile([128, 1], F16, tag="fc2w", name="fc2w_t")
        i64_t = pp.tile([C, C], F16, tag="i64", name="i64_t")
        is32_t = pp.tile([C, C], F32, tag="is32", name="is32_t")
        js32_t = pp.tile([C, C], F32, tag="js32", name="js32_t")

        nc.sync.dma_start(fc0w_t[:], fc0w_d[:])
        nc.sync.dma_start(fc0b_t[:], fc0b_d[:])
        for l in range(4):
            nc.sync.dma_start(cwt_t[l][:], cwt_d[l])
            nc.sync.dma_start(cb_t[l][:], cb_d[l])
        nc.sync.dma_start(fc1w_t[:], fc1w_d[:])
        nc.sync.dma_start(fc1b_t[:], fc1b_d[:])
        nc.sync.dma_start(fc2w_t[:], fc2w_d[:])
        nc.sync.dma_start(i64_t[:], i64_d[:])
        nc.sync.dma_start(is32_t[:], is32_d[:])
        nc.sync.dma_start(js32_t[:], js32_d[:])

        # ------------- V build + fc0 -------------
        with tc.tile_pool(name="vbuild", bufs=1) as vb, \
             tc.tile_pool(name="vbps", bufs=1, space="PSUM") as vbps:
            selT_t = vb.tile([49, 608], F32R, tag="selT", name="selT_t")
            nc.sync.dma_start(selT_t[:], selT_d[:])
            selInv_t = [vb.tile([49, 128], F32R, tag=f"si{k}", name=f"si{k}")
                        for k in range(6)]
            for k in range(6):
                nc.sync.dma_start(selInv_t[k][:], selInv_d[k])

            for b in range(BL):
                for c8 in range(8):
                    cols = slice(512 * c8, 512 * (c8 + 1))
                    ckt = vb.tile([49, 512], F32R, tag="ck", bufs=3, name=f"ck{b}_{c8}")
                    nc.sync.dma_start(ckt[:], ck_d[b, :, cols])

                    # fc0 for this chunk
                    ph0 = vbps.tile([C, 512], F32, tag="ph0", bufs=2, name=f"ph0_{b}_{c8}")
                    nc.tensor.matmul(ph0[:], fc0w_t[:], ckt[0:2, :],
                                     start=True, stop=True)
                    nc.scalar.activation(h[b][:, cols], ph0[:], AF.Identity,
                                         bias=fc0b_t[:, :])

                    # V^T slabs for the 4 n-subchunks of 128
                    for s in range(4):
                        pv = vbps.tile([128, 608], F32, tag="pv", bufs=2,
                                       name=f"pv{b}_{c8}_{s}")
                        lhs = ckt[:, 128 * s:128 * (s + 1)]
                        nc.tensor.matmul(pv[:, 0:512], lhs, selT_t[:, 0:512],
                                         start=True, stop=True)
                        nc.tensor.matmul(pv[:, 512:608], lhs, selT_t[:, 512:608],
                                         start=True, stop=True)
                        # range-reduce: Vi cols shift 0, Vr cols shift pi/2
                        nc.vector.add_range_wrap(pv[:, 0:304], pv[:, 0:304],
                                                 shift=0.0, bound=PI, period=2 * PI)
                        nc.vector.add_range_wrap(pv[:, 304:608], pv[:, 304:608],
                                                 shift=PI / 2, bound=PI, period=2 * PI)
                        nc.scalar.activation(vt[b][4 * c8 + s][:, :], pv[:, :], AF.Sin)

                    # V-inv tiles (m-part layout), this n-chunk
                    for k in range(6):
                        rows = vrows[k]
                        pq = vbps.tile([128, 512], F32, tag="pq", bufs=2,
                                       name=f"pq{b}_{c8}_{k}")
                        nc.tensor.matmul(pq[0:rows, :], selInv_t[k][:, 0:rows],
                                         ckt[:, :], start=True, stop=True)
                        nc.vector.add_range_wrap(pq[0:rows, :], pq[0:rows, :],
                                                 shift=(PI / 2 if k < 3 else 0.0),
                                                 bound=PI, period=2 * PI)
                        nc.scalar.activation(vinv[b][k][:, cols], pq[0:rows, :], AF.Sin)

        if DEBUG:
            for b in range(BL):
                nc.sync.dma_start(dbg['h0'][b], h[b][:])
            nc.sync.dma_start(dbg['vt0'][:], vt[0][0][:])
            nc.sync.dma_start(dbg['vi0'][:], vinv[0][0][:])

        # ------------- layers -------------
        with tc.tile_pool(name="work", bufs=1) as wk, \
             tc.tile_pool(name="wkps", bufs=1, space="PSUM") as wkps:

            for l in range(4):
                # ---- forward NUDFT ----
                px = wkps.tile([128, NWP], F32, tag="px", bufs=1, name=f"px{l}")
                for kt in range(32):
                    pt = wkps.tile([128, 2 * C], F16, tag="pt", bufs=2,
                                   name=f"pt{l}_{kt}")
                    for b in range(BL):
                        nc.tensor.matmul(pt[:, 32 * b:32 * (b + 1)],
                                         h[b][:, 128 * kt:128 * (kt + 1)],
                                         i64_t[:], start=True, stop=True,
                                         is_transpose=True)
                    hTt = wk.tile([128, 2 * C], F16, tag="hT", bufs=3,
                                  name=f"hT{l}_{kt}")
                    # fp16 PE-transpose ignores the stationary operand, so the
                    # 1/64 NUDFT pre-scale is applied here instead
                    nc.vector.tensor_scalar(hTt[:], pt[:], 1.0 / 64.0, None,
                                            op0=ALU.mult)
                    for g in range(4):
                        b, ri = g // 2, g % 2      # ri: 0 = real, 1 = imag
                        rhs = vt[b][kt][:, 304:608] if ri == 0 else vt[b][kt][:, 0:304]
                        nc.tensor.matmul(px[32 * g:32 * (g + 1), :],
                                         hTt[:, 32 * b:32 * (b + 1)], rhs,
                                         start=(kt == 0), stop=(kt == 31),
                                         tile_position=(0, 32 * g))

                # ---- R slab (mode-mix inputs), both batches interleaved ----
                R = wk.tile([128, 288], F16, tag="R", bufs=1, name=f"R{l}")
                R3 = R.rearrange("p (a s) -> p a s", s=12)
                for b in range(BL):
                    row_xr = 64 * b          # px rows: g = 2b + ri
                    row_xi = 64 * b + 32
                    for par in range(2):
                        out_r0 = 0 if par == 0 else 64
                        out_i0 = 32 if par == 0 else 96
                        # top + a=12 (direct): m = 23a + 2q + par, a in [0,12]
                        o_top = _cap(R3, 0, 32, [[12, 13], [2, 6]], b)
                        i_top = _cap(px, 0, 32, [[23, 13], [2, 6]], par)
                        nc.vector.tensor_copy(
                            _cap(R3, out_r0, 32, [[12, 13], [2, 6]], b),
                            _cap(px, row_xr, 32, [[23, 13], [2, 6]], par))
                        nc.vector.tensor_copy(
                            _cap(R3, out_i0, 32, [[12, 13], [2, 6]], b),
                            _cap(px, row_xi, 32, [[23, 13], [2, 6]], par))
                        # bot bulk (conj): a in [13,24), in col 576-23a-2q-par
                        nc.vector.tensor_copy(
                            _cap(R3, out_r0, 32, [[12, 11], [2, 6]], 156 + b),
                            _cap(px, row_xr, 32, [[-23, 11], [-2, 6]], 277 - par))
                        nc.vector.tensor_scalar(
                            _cap(R3, out_i0, 32, [[12, 11], [2, 6]], 156 + b),
                            _cap(px, row_xi, 32, [[-23, 11], [-2, 6]], 277 - par),
                            -1.0, None, op0=ALU.mult)
                        # fixups: s = a-12 (P-columns, direct, xi positive)
                        cnt = 5 if par == 0 else 6
                        s0 = 2 if par == 0 else 1
                        o_off = 13 * s0 + 144 - par + b
                        i_off = 288 + s0 - 1
                        nc.vector.tensor_copy(
                            _cap(R3, out_r0, 32, [[26, cnt]], o_off),
                            _cap(px, row_xr, 32, [[2, cnt]], i_off))
                        nc.vector.tensor_copy(
                            _cap(R3, out_i0, 32, [[26, cnt]], o_off),
                            _cap(px, row_xi, 32, [[2, cnt]], i_off))

                if DEBUG and l == 0:
                    nc.sync.dma_start(dbg['R'][:], R[:])

                # ---- mode mix: 144 rounds of [128,128] fp16 matmul ----
                pm = wkps.tile([128, 288], F32, tag="pm", bufs=1, name=f"pm{l}")
                for t36 in range(36):
                    mw = wk.tile([128, 512], F16, tag="mmw", bufs=2,
                                 name=f"mw{l}_{t36}")
                    nc.sync.dma_start(
                        mw.rearrange("p (r q) -> p r q", r=4),
                        mmw_d[l, 4 * t36:4 * (t36 + 1)].rearrange(
                            "r p q -> p r q"))
                    for rr in range(4):
                        r = 4 * t36 + rr
                        nc.tensor.matmul(pm[:, 2 * r:2 * r + 2],
                                         mw[:, 128 * rr:128 * (rr + 1)],
                                         R[:, 2 * r:2 * r + 2],
                                         start=True, stop=True)

                # ---- flat extraction + coefficient slabs ----
                frs = [wk.tile([C, NWP], F32, tag=f"frs{b}", name=f"frs{l}_{b}")
                       for b in range(BL)]
                fis = [wk.tile([C, NWP], F32, tag=f"fis{b}", name=f"fis{l}_{b}")
                       for b in range(BL)]
                frx = [wk.tile([C, NWP], F32, tag=f"frx{b}", name=f"frx{l}_{b}")
                       for b in range(BL)]
                fix = [wk.tile([C, NWP], F32, tag=f"fix{b}", name=f"fix{l}_{b}")
                       for b in range(BL)]
                for b in range(BL):
                    nc.vector.memset(frs[b][:, 288:NWP], 0.0)
                    nc.vector.memset(fis[b][:, 288:NWP], 0.0)
                    nc.vector.memset(frx[b][:], 0.0)
                    nc.vector.memset(fix[b][:], 0.0)
                    # even u from pm rows 0:32 (or) / 32:64 (oi), odd u from 64:96 / 96:128
                    nc.vector.tensor_copy(_cap(frs[b], 0, 32, [[2, 144]], 0),
                                          _cap(pm, 0, 32, [[2, 144]], b))
                    nc.vector.tensor_copy(_cap(frs[b], 0, 32, [[2, 144]], 1),
                                          _cap(pm, 64, 32, [[2, 144]], b))
                    nc.vector.tensor_copy(_cap(fis[b], 0, 32, [[2, 144]], 0),
                                          _cap(pm, 32, 32, [[2, 144]], b))
                    nc.vector.tensor_copy(_cap(fis[b], 0, 32, [[2, 144]], 1),
                                          _cap(pm, 96, 32, [[2, 144]], b))
                    # frx/fix: partner-coefficient slabs (read via rearranged views)
                    for (dst, src) in ((frx[b], frs[b]), (fix[b], fis[b])):
                        d3 = dst[:, 0:288].rearrange("p (j i) -> p j i", i=24)
                        s3 = src[:, 0:288].rearrange("p (j i) -> p j i", i=24)
                        nc.vector.tensor_copy(d3[:, 1:12, 1:12], s3[:, 1:12, 0:11])
                        nc.vector.tensor_copy(d3[:, 1:12, 13:24], s3[:, 1:12, 12:23])
                        nc.vector.tensor_copy(d3[:, 1:12, 0:1], s3[:, 1:12, 23:24])
                        # P columns: partner col 24*(23-j2)+11, j2 = 12..22
                        nc.vector.tensor_copy(dst[:, 288:299],
                                              s3[:, 11:0:-1, 11:12].rearrange(
                                                  "p j i -> p (j i)"))
                    nc.vector.tensor_scalar(fix[b][:, 288:299], fix[b][:, 288:299],
                                            -1.0, None, op0=ALU.mult)

                if DEBUG and l == 0:
                    nc.sync.dma_start(dbg['frs'][:], frs[0][:])
                    nc.sync.dma_start(dbg['frx'][:], frx[0][:])

                # ---- A^T / B^T via accumulate transposes ----
                cw3 = [128, 128, 48]
                AT = [[wk.tile([cw3[ch], C], F16, tag=f"AT{b}_{ch}",
                               name=f"AT{l}_{b}_{ch}") for ch in range(3)]
                      for b in range(BL)]
                BT = [[wk.tile([cw3[ch], C], F16, tag=f"BT{b}_{ch}",
                               name=f"BT{l}_{b}_{ch}") for ch in range(3)]
                      for b in range(BL)]
                for b in range(BL):
                    for ch in range(3):
                        cw_ = cw3[ch]
                        csl = slice(128 * ch, 128 * ch + cw_)
                        for (dstt, s_dir, s_flp) in ((AT[b][ch], frs[b], frx[b]),
                                                     (BT[b][ch], fis[b], fix[b])):
                            pc = wkps.tile([128, C], F32, tag="pc", bufs=1,
                                           name=f"pc{l}_{b}_{ch}_{0 if s_dir is frs[b] or s_dir is fis[b] else 1}")
                            nc.tensor.matmul(pc[0:cw_, :], s_dir[:, csl], is32_t[:],
                                             start=True, stop=False,
                                             is_transpose=True)
                            nc.tensor.matmul(pc[0:cw_, :], s_flp[:, csl], js32_t[:],
                                             start=False, stop=True,
                                             is_transpose=True)
                            # transpose-mode rhs magnitudes are not applied;
                            # the 1/32 coefficient scale happens here
                            nc.vector.tensor_scalar(dstt[:], pc[0:cw_, :],
                                                    1.0 / 32.0, None, op0=ALU.mult)

                if DEBUG and l == 0:
                    nc.sync.dma_start(dbg['AT0'][:], AT[0][0][:])

                # ---- inverse NUDFT + conv + activation ----
                last = (l == 3)
                for c8 in range(8):
                    cols = slice(512 * c8, 512 * (c8 + 1))
                    pi_ = wkps.tile([64, 512], F32, tag="pinv", bufs=2,
                                    name=f"pinv{l}_{c8}")
                    for b in range(BL):
                        sl = pi_[32 * b:32 * (b + 1), :]
                        tp = (0, 32 * b)
                        nc.tensor.matmul(sl, AT[b][0][:], vinv[b][0][:, cols],
                                         start=True, stop=False, tile_position=tp)
                        nc.tensor.matmul(sl, AT[b][1][:], vinv[b][1][:, cols],
                                         start=False, stop=False, tile_position=tp)
                        nc.tensor.matmul(sl, AT[b][2][:], vinv[b][2][:, cols],
                                         start=False, stop=False, tile_position=tp)
                        nc.tensor.matmul(sl, BT[b][0][:], vinv[b][3][:, cols],
                                         start=False, stop=False, tile_position=tp)
                        nc.tensor.matmul(sl, BT[b][1][:], vinv[b][4][:, cols],
                                         start=False, stop=False, tile_position=tp)
                        nc.tensor.matmul(sl, BT[b][2][:], vinv[b][5][:, cols],
                                         start=False, stop=False, tile_position=tp)
                        nc.tensor.matmul(sl, cwt_t[l][:], h[b][:, cols],
                                         start=False, stop=True, tile_position=tp)
                    for b in range(BL):
                        nc.scalar.activation(
                            h[b][:, cols], pi_[32 * b:32 * (b + 1), :],
                            AF.Identity if last else AF.Gelu,
                            bias=cb_t[l][:, :])

        if DEBUG:
            for b in range(BL):
                nc.sync.dma_start(dbg['h1'][b], h[b][:])
        # ---- head: fc1 + gelu + fc2 ----
        with tc.tile_pool(name="head", bufs=1) as hd, \
             tc.tile_pool(name="hdps", bufs=1, space="PSUM") as hdps:
            for b in range(BL):
                for c8 in range(8):
                    cols = slice(512 * c8, 512 * (c8 + 1))
                    pg = hdps.tile([128, 512], F32, tag="pg", bufs=2,
                                   name=f"pg{b}_{c8}")
                    nc.tensor.matmul(pg[:], fc1w_t[:], h[b][:, cols],
                                     start=True, stop=True)
                    g = hd.tile([128, 512], F16, tag="g", bufs=2, name=f"g{b}_{c8}")
                    nc.scalar.activation(g[:], pg[:], AF.Gelu, bias=fc1b_t[:, :])
                    py = hdps.tile([1, 512], F32, tag="py", bufs=2,
                                   name=f"py{b}_{c8}")
                    nc.tensor.matmul(py[:], fc2w_t[:], g[:], start=True, stop=True)
                    ys = hd.tile([1, 512], F32, tag="ys", bufs=2, name=f"ys{b}_{c8}")
                    nc.scalar.activation(ys[:], py[:], AF.Copy)
                    nc.sync.dma_start(y_d[b:b + 1, cols], ys[:])

    nc.compile()
    return nc


# --------------------------------------------------------------------------
# host marshaling
# --------------------------------------------------------------------------
def _marshal(pos, fc0_w, fc0_b, sw1r, sw1i, sw2r, sw2i, cw, cb,
             fc1_w, fc1_b, fc2_w, fc2_b):
    xp = (pos[:, :, 0] - pos[:, :, 0].min()).astype(np.float64)
    yp = (pos[:, :, 1] - pos[:, :, 1].min()).astype(np.float64)
    sx = np.float64(np.float32(6.28) / np.float32(xp.max()))
    sy = np.float64(np.float32(6.28) / np.float32(yp.max()))
    kx = np.concatenate([np.arange(MODES), np.arange(-MODES, 0)]).astype(np.float64)
    ky = np.concatenate([np.arange(MODES), np.arange(-(MODES - 1), 0)]).astype(np.float64)

    def wrap(v):
        return v - 2 * np.pi * np.round(v / (2 * np.pi))

    ck = np.zeros((B, 49, N), np.float32)
    ck[:, 0, :] = xp.astype(np.float32)
    ck[:, 1, :] = yp.astype(np.float32)
    for i in range(24):
        ck[:, 2 + i, :] = wrap(kx[i] * sx * xp).astype(np.float32)
    for j in range(23):
        ck[:, 26 + j, :] = wrap(ky[j] * sy * yp).astype(np.float32)

    worder = _w_rows()
    # selT [49, 608]: cols 0:304 = -phase (Vi), 304:608 = +phase (Vr)
    selT = np.zeros((49, 608), np.float32)
    for w, m in enumerate(worder):
        i, j = m % 24, m // 24
        selT[2 + i, w] = -1.0
        selT[26 + j, w] = -1.0
        selT[2 + i, 304 + w] = 1.0
        selT[26 + j, 304 + w] = 1.0
    # selInv [6, 49, 128]: tiles 0..2 = Vr chunks (+phase), 3..5 = Vi (-phase)
    selInv = np.zeros((6, 49, 128), np.float32)
    for k in range(6):
        sgn = 1.0 if k < 3 else -1.0
        ch = k % 3
        for p in range(128):
            w = 128 * ch + p
            if w >= NW:
                break
            m = worder[w]
            i, j = m % 24, m // 24
            selInv[k, 2 + i, p] = sgn
            selInv[k, 26 + j, p] = sgn

    # mode-mix weights, augmented 2-mode block-diagonal
    mmw = np.zeros((4, 144, 128, 128), np.float16)
    for l in range(4):
        w1 = sw1r[l].astype(np.float64) + 1j * sw1i[l].astype(np.float64)
        w2 = sw2r[l].astype(np.float64) + 1j * sw2i[l].astype(np.float64)
        for u in range(288):
            a, s = u // 12, u % 12
            wm = w1[:, :, a, s] if a < 12 else w2[:, :, a - 12, s]
            wr = wm.real.astype(np.float16)
            wi = wm.imag.astype(np.float16)
            r, half = u // 2, 64 * (u % 2)
            mmw[l, r, half:half + 32, half:half + 32] = wr
            mmw[l, r, half + 32:half + 64, half:half + 32] = -wi
            mmw[l, r, half:half + 32, half + 32:half + 64] = wi
            mmw[l, r, half + 32:half + 64, half + 32:half + 64] = wr

    cwt = np.ascontiguousarray(cw.transpose(0, 2, 1)).astype(np.float16)  # [l, c_in, c_out]
    cbm = cb.reshape(4, C, 1).astype(np.float32)

    eye = np.eye(C, dtype=np.float32)
    args = dict(
        selT=selT, selInv=selInv,
        fc0w=fc0_w.astype(np.float32), fc0b=fc0_b.reshape(C, 1).astype(np.float32),
        mmw=mmw, cwt=cwt, cb=cbm,
        fc1w=fc1_w.astype(np.float16), fc1b=fc1_b.reshape(128, 1).astype(np.float32),
        fc2w=fc2_w.reshape(128, 1).astype(np.float16),
        i64=eye.astype(np.float16),
        is32=eye.astype(np.float32),
        js32=eye[::-1].copy().astype(np.float32),
    )
    return ck, args


def kernel(**inputs):
    pos = np.asarray(inputs['pos'])
    ck, shared = _marshal(**{k: np.asarray(v) for k, v in inputs.items()})

    if 'nc' not in _CACHE:
        _CACHE['nc'] = _build_program()
    nc = _CACHE['nc']

    in_maps = []
    for core in range(NCORES):
        m = dict(shared)
        m['ck'] = ck[BL * core:BL * (core + 1)]
        in_maps.append(m)

    res = run_bass_kernel_spmd(nc, in_maps, list(range(NCORES)), trace=TRACE)
    _CACHE['last_results'] = res

    fc2_b = np.asarray(inputs['fc2_b']).astype(np.float32)
    out = np.zeros((B, N, 1), np.float32)
    for core in range(NCORES):
        out[BL * core:BL * (core + 1), :, 0] = res.results[core]['y']
    out += fc2_b.reshape(1, 1, 1)
    return out



# revision 19
# speedup vs baseline: 1.6844x; 1.6844x over previous
"""Trainium2 Bass kernel for the FNO-SMM problem (nn_FNO_SMM_34488587387600), v2.

Data-parallel over 8 NeuronCores: 2 batches per core. Per core:
  - V build: selection matmuls (host-prewrapped per-k angle tables) -> f16
    angle sums in PSUM -> DVE range-wrap -> one ACT Sin per 2-chunk group
    writes vt (n-major [n, cos 0:304 | -sin 304:608]); vinv (m-major, 5
    packed 128-row tiles) derived from vt by PE transposes + batched copies.
  - 4 spectral layers: forward NUDFT (col-tiled f16 matmuls into PSUM px),
    mode mix as 288 compact [64,64] augmented-complex matmuls reading x_ft
    columns directly (conjugate folding baked into weights host-side,
    weights streamed as 8 large contiguous DMAs per layer), coefficient
    extraction + packed coefficient slabs, inverse NUDFT fused with the
    1x1 conv, exact-erf gelu.
  - fc1/fc2 head.
"""
import sys
import os

sys.path.insert(0, '/opt/trn_rl_repo')

import numpy as np
from contextlib import ExitStack

import concourse.bass as bass
import concourse.tile as tile
from concourse import bacc, mybir
from concourse.bass_utils import run_bass_kernel_spmd

MODES = 12
C = 32
N = 4096
B = 16
NCORES = 8
BL = B // NCORES          # 2 batches per core
NW = 299                  # working-set rows: 288 + 11 unpaired (kx=-12, ky<0)
NWP = 304                 # padded
PI = float(np.pi)

F32 = mybir.dt.float32
F32R = mybir.dt.float32r
F16 = mybir.dt.float16
AF = mybir.ActivationFunctionType
ALU = mybir.AluOpType

TRACE = False
DEBUG = False

_CACHE = {}


# --------------------------------------------------------------------------
# host-side index helpers (python ints only; used at build/marshal time)
# --------------------------------------------------------------------------
def _w_rows():
    """W-set V-row indices: m in [0,288) then the 11 unpaired rows."""
    return list(range(288)) + [24 * j + 12 for j in range(12, 23)]


def mode_col(u):
    """px/xs column + conj flag for mode u = 12a + s."""
    a, s = divmod(u, 12)
    f = 23 * a + s
    if f < 288:
        return f, False
    i, j = f % 24, f // 24
    if i == 12:
        return 288 + (j - 12), False
    return 24 * (23 - j) + ((24 - i) % 24), True


def _cap(t_ap, row0, nrows, pairs, free_off):
    """Custom AP on a tile's underlying tensor: rows [row0, row0+nrows),
    free pattern `pairs` ([[step, count], ...]) at element offset free_off."""
    base = t_ap.ap
    pstep = base[0][0]
    return bass.AP(tensor=t_ap.tensor, offset=row0 * pstep + free_off + t_ap.offset,
                   ap=[[pstep, nrows]] + [list(p) for p in pairs])


# --------------------------------------------------------------------------
# device program
# --------------------------------------------------------------------------
def _build_program():
    nc = bacc.Bacc("TRN2", target_bir_lowering=False, debug=False,
                   num_devices=NCORES)

    din = {}
    def dram_in(name, shape, dt):
        din[name] = nc.dram_tensor(name, list(shape), dt, kind="ExternalInput").ap()
        return din[name]

    ck_d = dram_in('ck', [BL, 50, N], F32R)
    selT_d = dram_in('selT', [50, 608], F32R)
    mmw_d = dram_in('mmw2', [4, 2, 64, 9216], F16)
    fc0w_d = dram_in('fc0w', [2, C], F32R)
    fc0b_d = dram_in('fc0b', [C, 1], F32)
    cwt_d = dram_in('cwt', [4, C, C], F16)
    cb_d = dram_in('cb', [4, C, 1], F32)
    fc1w_d = dram_in('fc1w', [C, 128], F16)
    fc1b_d = dram_in('fc1b', [128, 1], F32)
    fc2w_d = dram_in('fc2w', [128, 1], F16)
    i64_d = dram_in('i64', [C, C], F16)
    i128_d = dram_in('i128', [128, 128], F16)
    is32_d = dram_in('is32', [C, C], F32)
    js32_d = dram_in('js32', [C, C], F32)

    y_d = nc.dram_tensor('y', [BL, N], F32, kind="ExternalOutput").ap()
    dbg = {}
    if DEBUG:
        dbg['h0'] = nc.dram_tensor('dbg_h0', [BL, C, N], F16, kind="ExternalOutput").ap()
        dbg['vt0'] = nc.dram_tensor('dbg_vt0', [128, 608], F16, kind="ExternalOutput").ap()
        dbg['vi0'] = nc.dram_tensor('dbg_vi0', [128, 512], F16, kind="ExternalOutput").ap()
        dbg['xs0'] = nc.dram_tensor('dbg_xs0', [64, 608], F16, kind="ExternalOutput").ap()
        dbg['pm0'] = nc.dram_tensor('dbg_pm0', [64, 576], F32, kind="ExternalOutput").ap()
        dbg['frs'] = nc.dram_tensor('dbg_frs', [C, NWP], F32, kind="ExternalOutput").ap()
        dbg['CT0'] = nc.dram_tensor('dbg_CT0', [128, C], F16, kind="ExternalOutput").ap()
        dbg['h1'] = nc.dram_tensor('dbg_h1', [BL, C, N], F16, kind="ExternalOutput").ap()

    mcols = [mode_col(u)[0] for u in range(288)]

    with tile.TileContext(nc) as tc, ExitStack() as ctx:
        # ------------- persistent pool -------------
        pp = ctx.enter_context(tc.tile_pool(name="persist", bufs=1))
        vt = [pp.tile([128, 32 * 640], F16, tag=f"vt{b}", name=f"vt{b}")
              for b in range(BL)]
        vinv = [pp.tile([128, 5 * 4096], F16, tag=f"vi{b}", name=f"vi{b}")
                for b in range(BL)]
        h = [pp.tile([C, N], F16, tag=f"h{b}", name=f"h{b}") for b in range(BL)]

        fc0w_t = pp.tile([2, C], F32R, tag="fc0w", name="fc0w_t")
        fc0b_t = pp.tile([C, 1], F32, tag="fc0b", name="fc0b_t")
        cwt_t = [pp.tile([C, C], F16, tag=f"cwt{l}", name=f"cwt{l}") for l in range(4)]
        cb_t = [pp.tile([C, 1], F32, tag=f"cb{l}", name=f"cb{l}") for l in range(4)]
        fc1w_t = pp.tile([C, 128], F16, tag="fc1w", name="fc1w_t")
        fc1b_t = pp.tile([128, 1], F32, tag="fc1b", name="fc1b_t")
        fc2w_t = pp.tile([128, 1], F16, tag="fc2w", name="fc2w_t")
        i64_t = pp.tile([C, C], F16, tag="i64", name="i64_t")
        i128_t = pp.tile([128, 128], F16, tag="i128", name="i128_t")
        is32_t = pp.tile([C, C], F32, tag="is32", name="is32_t")
        js32_t = pp.tile([C, C], F32, tag="js32", name="js32_t")

        nc.sync.dma_start(fc0w_t[:], fc0w_d[:])
        nc.sync.dma_start(fc0b_t[:], fc0b_d[:])
        for l in range(4):
            nc.sync.dma_start(cwt_t[l][:], cwt_d[l])
            nc.sync.dma_start(cb_t[l][:], cb_d[l])
        nc.sync.dma_start(fc1w_t[:], fc1w_d[:])
        nc.sync.dma_start(fc1b_t[:], fc1b_d[:])
        nc.sync.dma_start(fc2w_t[:], fc2w_d[:])
        nc.sync.dma_start(i64_t[:], i64_d[:])
        nc.sync.dma_start(i128_t[:], i128_d[:])
        nc.sync.dma_start(is32_t[:], is32_d[:])
        nc.sync.dma_start(js32_t[:], js32_d[:])

        # ------------- V build + fc0 -------------
        with tc.tile_pool(name="vbuild", bufs=1) as vb, \
             tc.tile_pool(name="vbps", bufs=1, space="PSUM") as vbps:
            selT_t = vb.tile([50, 608], F32R, tag="selT", name="selT_t")
            nc.sync.dma_start(selT_t[:], selT_d[:])

            # zero the 16-col pads of the [cos 304|z16|sin 304|z16] kt-blocks
            for b in range(BL):
                nc.vector.memset(
                    _cap(vt[b], 0, 128, [[320, 64], [1, 16]], 304), 0.0)

            cp_eng = 0
            for b in range(BL):
                for c8 in range(8):
                    cols = slice(512 * c8, 512 * (c8 + 1))
                    ckt = vb.tile([50, 512], F32R, tag="ck", bufs=2,
                                  name=f"ck{b}_{c8}")
                    nc.sync.dma_start(ckt[:], ck_d[b, :, cols])

                    ph0 = vbps.tile([C, 512], F32, tag="ph0", bufs=2,
                                    name=f"ph0_{b}_{c8}")
                    nc.tensor.matmul(ph0[:], fc0w_t[:], ckt[0:2, :],
                                     start=True, stop=True)
                    nc.scalar.activation(h[b][:, cols], ph0[:], AF.Identity,
                                         bias=fc0b_t[:, :])

                    for s in range(4):
                        kt = 4 * c8 + s
                        pv = vbps.tile([128, 640], F32, tag="pv", bufs=2,
                                       name=f"pv{b}_{kt}")
                        nc.tensor.matmul(pv[:, 0:512],
                                         ckt[:, 128 * s:128 * (s + 1)],
                                         selT_t[:, 0:512], start=True, stop=True)
                        nc.tensor.matmul(pv[:, 512:608],
                                         ckt[:, 128 * s:128 * (s + 1)],
                                         selT_t[:, 512:608], start=True, stop=True)
                        # pi/2 cos-shift comes in via the const ck row
                        nc.vector.add_range_wrap(pv[:, 0:608], pv[:, 0:608],
                                                 shift=0.0, bound=PI,
                                                 period=2 * PI)
                        nc.scalar.activation(
                            _cap(vt[b], 0, 128, [[320, 2], [1, 304]], 640 * kt),
                            pv[:, 0:608], AF.Sin)
                        tp = vbps.tile([128, 640], F16, tag="tp", bufs=2,
                                       name=f"tp{b}_{kt}")
                        # packed-640 rows: [cos 0:304 | z16 | sin 0:304 | z16]
                        for t in range(5):
                            nc.tensor.matmul(
                                tp[:, 128 * t:128 * t + 128],
                                vt[b][:, 640 * kt + 128 * t:
                                       640 * kt + 128 * (t + 1)],
                                i128_t[:], start=True, stop=True,
                                is_transpose=True)
                        dst = _cap(vinv[b], 0, 128, [[4096, 5], [1, 128]],
                                   128 * kt)
                        if cp_eng == 1:
                            nc.scalar.activation(dst, tp[:, :], AF.Copy)
                        else:
                            nc.vector.tensor_copy(dst, tp[:, :])
                        cp_eng = (cp_eng + 1) % 2

        if DEBUG:
            for b in range(BL):
                nc.sync.dma_start(dbg['h0'][b], h[b][:])
            nc.sync.dma_start(dbg['vt0'][:], vt[0][:, 0:608])
            nc.sync.dma_start(dbg['vi0'][:], vinv[0][:, 0:512])

        # ------------- layers -------------
        with tc.tile_pool(name="work", bufs=1) as wk, \
             tc.tile_pool(name="wkps", bufs=1, space="PSUM") as wkps:

            CT = [[wk.tile([128, C], F16, tag=f"CT{b}_{t}",
                           name=f"CT{b}_{t}") for t in range(5)]
                  for b in range(BL)]

            for l in range(4):
                last = (l == 3)
                # ---- forward NUDFT ----
                px = wkps.tile([128, NWP], F32, tag="px", bufs=1, name=f"px{l}")
                for kt in range(32):
                    pt = wkps.tile([128, 2 * C], F16, tag="pt", bufs=2,
                                   name=f"pt{l}_{kt}")
                    for b in range(BL):
                        nc.tensor.matmul(pt[:, 32 * b:32 * (b + 1)],
                                         h[b][:, 128 * kt:128 * (kt + 1)],
                                         i64_t[:], start=True, stop=True,
                                         is_transpose=True)
                    hTt = wk.tile([128, 2 * C], F16, tag="hT", bufs=3,
                                  name=f"hT{l}_{kt}")
                    nc.vector.tensor_copy(hTt[:], pt[:])
                    for g in range(4):
                        b, ri = g // 2, g % 2
                        rhs = vt[b][:, 640 * kt + 320 * ri:
                                    640 * kt + 320 * ri + 304]
                        nc.tensor.matmul(px[32 * g:32 * (g + 1), :],
                                         hTt[:, 32 * b:32 * (b + 1)], rhs,
                                         start=(kt == 0), stop=(kt == 31),
                                         tile_position=(0, 32 * g))

                # ---- x_ft slab: xs2[:, 2w+b] = px[64b:64b+64, w] ----
                xs2 = wk.tile([64, 2 * NWP], F16, tag="xs2", bufs=1,
                              name=f"xs2_{l}")
                for b in range(BL):
                    nc.vector.tensor_copy(
                        _cap(xs2, 0, 64, [[2, NWP]], b),
                        px[64 * b:64 * (b + 1), :])

                if DEBUG and l == 0:
                    nc.sync.dma_start(dbg['xs0'][:], xs2[:])

                # ---- mode mix: 288 compact [64,64] aug matmuls ----
                pm = wkps.tile([64, 576], F32, tag="pm", bufs=1, name=f"pm{l}")
                for c4 in range(4):
                    sE = wk.tile([64, 2304], F16, tag="sE", bufs=2,
                                 name=f"sE{l}_{c4}")
                    sO = wk.tile([64, 2304], F16, tag="sO", bufs=2,
                                 name=f"sO{l}_{c4}")
                    nc.sync.dma_start(sE[:], mmw_d[l, 0, :,
                                                   2304 * c4:2304 * (c4 + 1)])
                    nc.sync.dma_start(sO[:], mmw_d[l, 1, :,
                                                   2304 * c4:2304 * (c4 + 1)])
                    for rr in range(36):
                        r = 36 * c4 + rr
                        for par in range(2):
                            u = 2 * r + par
                            mc = mcols[u]
                            st = sE if par == 0 else sO
                            nc.tensor.matmul(pm[:, 2 * u:2 * u + 2],
                                             st[:, 64 * rr:64 * (rr + 1)],
                                             xs2[:, 2 * mc:2 * mc + 2],
                                             start=True, stop=True)

                if DEBUG and l == 0:
                    nc.sync.dma_start(dbg['pm0'][:], pm[:])

                # ---- coefficient extraction ----
                frs = [wk.tile([C, NWP], F32, tag=f"frs{b}", name=f"frs{l}_{b}")
                       for b in range(BL)]
                fis = [wk.tile([C, NWP], F32, tag=f"fis{b}", name=f"fis{l}_{b}")
                       for b in range(BL)]
                frx = [wk.tile([C, NWP], F32, tag=f"frx{b}", name=f"frx{l}_{b}")
                       for b in range(BL)]
                fix = [wk.tile([C, NWP], F32, tag=f"fix{b}", name=f"fix{l}_{b}")
                       for b in range(BL)]
                for b in range(BL):
                    nc.vector.memset(frs[b][:, 288:NWP], 0.0)
                    nc.vector.memset(fis[b][:, 288:NWP], 0.0)
                    nc.vector.memset(frx[b][:], 0.0)
                    nc.vector.memset(fix[b][:], 0.0)
                    nc.vector.tensor_copy(frs[b][:, 0:288],
                                          _cap(pm, 0, 32, [[2, 288]], b))
                    nc.vector.tensor_copy(fis[b][:, 0:288],
                                          _cap(pm, 32, 32, [[2, 288]], b))
                    # partner-coefficient slabs
                    for (dst, src) in ((frx[b], frs[b]), (fix[b], fis[b])):
                        d3 = dst[:, 0:288].rearrange("p (j i) -> p j i", i=24)
                        s3 = src[:, 0:288].rearrange("p (j i) -> p j i", i=24)
                        nc.vector.tensor_copy(d3[:, 1:12, 1:12], s3[:, 1:12, 0:11])
                        nc.vector.tensor_copy(d3[:, 1:12, 13:24], s3[:, 1:12, 12:23])
                        nc.vector.tensor_copy(d3[:, 1:12, 0:1], s3[:, 1:12, 23:24])
                        nc.vector.tensor_copy(dst[:, 288:299],
                                              s3[:, 11:0:-1, 11:12].rearrange(
                                                  "p j i -> p (j i)"))
                    nc.vector.tensor_scalar(fix[b][:, 288:299], fix[b][:, 288:299],
                                            -1.0, None, op0=ALU.mult)

                if DEBUG and l == 0:
                    nc.sync.dma_start(dbg['frs'][:], frs[0][:])

                # ---- packed coefficient slabs CT[b][t] ----
                # t-tile sources: (kind, col0, width, ctrow0)
                tspec = [[(0, 0, 128, 0)], [(0, 128, 128, 0)],
                         [(0, 256, 48, 0), (1, 0, 64, 64)],
                         [(1, 64, 128, 0)], [(1, 192, 112, 0)]]
                for b in range(BL):
                    if l == 0:
                        nc.vector.memset(CT[b][2][32:64, :], 0.0)
                        nc.vector.memset(CT[b][4][96:128, :], 0.0)
                    for t in range(5):
                        for (kind, c0, wdt, r0) in tspec[t]:
                            sd = frs[b] if kind == 0 else fis[b]
                            sf = frx[b] if kind == 0 else fix[b]
                            pc = wkps.tile([128, C], F32, tag="pc", bufs=1,
                                           name=f"pc{l}_{b}_{t}_{r0}")
                            nc.tensor.matmul(pc[0:wdt, :], sd[:, c0:c0 + wdt],
                                             is32_t[:], start=True, stop=False,
                                             is_transpose=True)
                            nc.tensor.matmul(pc[0:wdt, :], sf[:, c0:c0 + wdt],
                                             js32_t[:], start=False, stop=True,
                                             is_transpose=True)
                            nc.vector.tensor_scalar(CT[b][t][r0:r0 + wdt, :],
                                                    pc[0:wdt, :],
                                                    1.0 / 2048.0, None,
                                                    op0=ALU.mult)

                if DEBUG and l == 0:
                    nc.sync.dma_start(dbg['CT0'][:], CT[0][0][:])

                # ---- inverse NUDFT + conv + activation ----
                for c8 in range(8):
                    cols = slice(512 * c8, 512 * (c8 + 1))
                    pi_ = wkps.tile([64, 512], F32, tag="pinv", bufs=2,
                                    name=f"pinv{l}_{c8}")
                    for b in range(BL):
                        sl = pi_[32 * b:32 * (b + 1), :]
                        tp_ = (0, 32 * b)
                        for t in range(5):
                            nc.tensor.matmul(
                                sl, CT[b][t][:],
                                vinv[b][:, 4096 * t + 512 * c8:
                                        4096 * t + 512 * (c8 + 1)],
                                start=(t == 0), stop=False, tile_position=tp_)
                        nc.tensor.matmul(sl, cwt_t[l][:], h[b][:, cols],
                                         start=False, stop=True,
                                         tile_position=tp_)
                    for b in range(BL):
                        nc.scalar.activation(
                            h[b][:, cols], pi_[32 * b:32 * (b + 1), :],
                            AF.Identity if last else AF.Gelu,
                            bias=cb_t[l][:, :])

        if DEBUG:
            for b in range(BL):
                nc.sync.dma_start(dbg['h1'][b], h[b][:])
        # ---- head: fc1 + gelu + fc2 ----
        with tc.tile_pool(name="head", bufs=1) as hd, \
             tc.tile_pool(name="hdps", bufs=1, space="PSUM") as hdps:
            for b in range(BL):
                for c8 in range(8):
                    cols = slice(512 * c8, 512 * (c8 + 1))
                    pg = hdps.tile([128, 512], F32, tag="pg", bufs=2,
                                   name=f"pg{b}_{c8}")
                    nc.tensor.matmul(pg[:], fc1w_t[:], h[b][:, cols],
                                     start=True, stop=True)
                    g = hd.tile([128, 512], F16, tag="g", bufs=2, name=f"g{b}_{c8}")
                    nc.scalar.activation(g[:], pg[:], AF.Gelu, bias=fc1b_t[:, :])
                    py = hdps.tile([1, 512], F32, tag="py", bufs=2,
                                   name=f"py{b}_{c8}")
                    nc.tensor.matmul(py[:], fc2w_t[:], g[:], start=True, stop=True)
                    ys = hd.tile([1, 512], F32, tag="ys", bufs=2, name=f"ys{b}_{c8}")
                    nc.vector.tensor_copy(ys[:], py[:])
                    nc.sync.dma_start(y_d[b:b + 1, cols], ys[:])

    nc.compile()
    return nc


# --------------------------------------------------------------------------
# host marshaling
# --------------------------------------------------------------------------
def _marshal(pos, fc0_w, fc0_b, sw1r, sw1i, sw2r, sw2i, cw, cb,
             fc1_w, fc1_b, fc2_w, fc2_b):
    xp = (pos[:, :, 0] - pos[:, :, 0].min()).astype(np.float64)
    yp = (pos[:, :, 1] - pos[:, :, 1].min()).astype(np.float64)
    sx = np.float64(np.float32(6.28) / np.float32(xp.max()))
    sy = np.float64(np.float32(6.28) / np.float32(yp.max()))
    kx = np.concatenate([np.arange(MODES), np.arange(-MODES, 0)]).astype(np.float64)
    ky = np.concatenate([np.arange(MODES), np.arange(-(MODES - 1), 0)]).astype(np.float64)

    def wrap(v):
        return v - 2 * np.pi * np.round(v / (2 * np.pi))

    ck = np.zeros((B, 50, N), np.float32)
    ck[:, 0, :] = xp.astype(np.float32)
    ck[:, 1, :] = yp.astype(np.float32)
    for i in range(24):
        ck[:, 2 + i, :] = wrap(kx[i] * sx * xp).astype(np.float32)
    for j in range(23):
        ck[:, 26 + j, :] = wrap(ky[j] * sy * yp).astype(np.float32)
    ck[:, 49, :] = np.float32(np.pi / 2)

    worder = _w_rows()
    # selT [50, 608]: cols 0:304 = +phase+pi/2 (cos), 304:608 = -phase (-sin)
    selT = np.zeros((50, 608), np.float32)
    for w, m in enumerate(worder):
        i, j = m % 24, m // 24
        selT[2 + i, w] = 1.0
        selT[26 + j, w] = 1.0
        selT[49, w] = 1.0
        selT[2 + i, 304 + w] = -1.0
        selT[26 + j, 304 + w] = -1.0

    # compact augmented mode-mix weights with conj baked in
    mmw2 = np.zeros((4, 2, 64, 9216), np.float16)
    for l in range(4):
        w1 = sw1r[l].astype(np.float64) + 1j * sw1i[l].astype(np.float64)
        w2 = sw2r[l].astype(np.float64) + 1j * sw2i[l].astype(np.float64)
        for u in range(288):
            a, s = u // 12, u % 12
            wm = w1[:, :, a, s] if a < 12 else w2[:, :, a - 12, s]
            wr = wm.real.astype(np.float16)
            wi = wm.imag.astype(np.float16)
            _, cj = mode_col(u)
            r, par = u // 2, u % 2
            blk = mmw2[l, par, :, 64 * r:64 * (r + 1)]
            blk[0:32, 0:32] = wr
            blk[0:32, 32:64] = wi
            if cj:
                blk[32:64, 0:32] = wi
                blk[32:64, 32:64] = -wr
            else:
                blk[32:64, 0:32] = -wi
                blk[32:64, 32:64] = wr

    cwt = np.ascontiguousarray(cw.transpose(0, 2, 1)).astype(np.float16)
    cbm = cb.reshape(4, C, 1).astype(np.float32)

    eye32 = np.eye(C, dtype=np.float32)
    args = dict(
        selT=selT, mmw2=mmw2,
        fc0w=fc0_w.astype(np.float32), fc0b=fc0_b.reshape(C, 1).astype(np.float32),
        cwt=cwt, cb=cbm,
        fc1w=fc1_w.astype(np.float16), fc1b=fc1_b.reshape(128, 1).astype(np.float32),
        fc2w=fc2_w.reshape(128, 1).astype(np.float16),
        i64=eye32.astype(np.float16),
        i128=np.eye(128, dtype=np.float16),
        is32=eye32,
        js32=eye32[::-1].copy(),
    )
    return ck, args


def kernel(**inputs):
    pos = np.asarray(inputs['pos'])
    ck, shared = _marshal(**{k: np.asarray(v) for k, v in inputs.items()})

    if 'nc' not in _CACHE:
        _CACHE['nc'] = _build_program()
    nc = _CACHE['nc']

    in_maps = []
    for core in range(NCORES):
        m = dict(shared)
        m['ck'] = ck[BL * core:BL * (core + 1)]
        in_maps.append(m)

    res = run_bass_kernel_spmd(nc, in_maps, list(range(NCORES)), trace=TRACE)
    _CACHE['last_results'] = res

    fc2_b = np.asarray(inputs['fc2_b']).astype(np.float32)
    out = np.zeros((B, N, 1), np.float32)
    for core in range(NCORES):
        out[BL * core:BL * (core + 1), :, 0] = res.results[core]['y']
    out += fc2_b.reshape(1, 1, 1)
    return out


# revision 27
# speedup vs baseline: 1.8385x; 1.0915x over previous
"""Trainium2 Bass kernel for the FNO-SMM problem (nn_FNO_SMM_34488587387600), v2.

Data-parallel over 8 NeuronCores: 2 batches per core. Per core:
  - V build: selection matmuls (host-prewrapped per-k angle tables) -> f16
    angle sums in PSUM -> DVE range-wrap -> one ACT Sin per 2-chunk group
    writes vt (n-major [n, cos 0:304 | -sin 304:608]); vinv (m-major, 5
    packed 128-row tiles) derived from vt by PE transposes + batched copies.
  - 4 spectral layers: forward NUDFT (col-tiled f16 matmuls into PSUM px),
    mode mix as 288 compact [64,64] augmented-complex matmuls reading x_ft
    columns directly (conjugate folding baked into weights host-side,
    weights streamed as 8 large contiguous DMAs per layer), coefficient
    extraction + packed coefficient slabs, inverse NUDFT fused with the
    1x1 conv, exact-erf gelu.
  - fc1/fc2 head.
"""
import sys
import os

sys.path.insert(0, '/opt/trn_rl_repo')

import numpy as np
from contextlib import ExitStack

import concourse.bass as bass
import concourse.tile as tile
from concourse import bacc, mybir
from concourse.bass_utils import run_bass_kernel_spmd

MODES = 12
C = 32
N = 4096
B = 16
NCORES = 8
BL = B // NCORES          # 2 batches per core
NW = 299                  # working-set rows: 288 + 11 unpaired (kx=-12, ky<0)
NWP = 304                 # padded
PI = float(np.pi)

F32 = mybir.dt.float32
F32R = mybir.dt.float32r
F16 = mybir.dt.float16
AF = mybir.ActivationFunctionType
ALU = mybir.AluOpType

TRACE = False
DEBUG = False

_CACHE = {}


# --------------------------------------------------------------------------
# host-side index helpers (python ints only; used at build/marshal time)
# --------------------------------------------------------------------------
def _w_rows():
    """W-set V-row indices: m in [0,288) then the 11 unpaired rows."""
    return list(range(288)) + [24 * j + 12 for j in range(12, 23)]


def mode_col(u):
    """px/xs column + conj flag for mode u = 12a + s."""
    a, s = divmod(u, 12)
    f = 23 * a + s
    if f < 288:
        return f, False
    i, j = f % 24, f // 24
    if i == 12:
        return 288 + (j - 12), False
    return 24 * (23 - j) + ((24 - i) % 24), True


def _cap(t_ap, row0, nrows, pairs, free_off):
    """Custom AP on a tile's underlying tensor: rows [row0, row0+nrows),
    free pattern `pairs` ([[step, count], ...]) at element offset free_off."""
    base = t_ap.ap
    pstep = base[0][0]
    return bass.AP(tensor=t_ap.tensor, offset=row0 * pstep + free_off + t_ap.offset,
                   ap=[[pstep, nrows]] + [list(p) for p in pairs])


# --------------------------------------------------------------------------
# device program
# --------------------------------------------------------------------------
def _build_program():
    nc = bacc.Bacc("TRN2", target_bir_lowering=False, debug=False,
                   num_devices=NCORES)

    din = {}
    def dram_in(name, shape, dt):
        din[name] = nc.dram_tensor(name, list(shape), dt, kind="ExternalInput").ap()
        return din[name]

    ck_d = dram_in('ck', [BL, 50, N], F32R)
    selT_d = dram_in('selT', [50, 608], F32R)
    mmw_d = dram_in('mmw2', [4, 2, 64, 9216], F16)
    fc0w_d = dram_in('fc0w', [2, C], F32R)
    fc0b_d = dram_in('fc0b', [C, 1], F32)
    cwt_d = dram_in('cwt', [4, C, C], F16)
    cb_d = dram_in('cb', [4, C, 1], F32)
    fc1w_d = dram_in('fc1w', [C, 128], F16)
    fc1b_d = dram_in('fc1b', [128, 1], F32)
    fc2w_d = dram_in('fc2w', [128, 1], F16)
    i64_d = dram_in('i64', [C, C], F16)
    i128_d = dram_in('i128', [128, 128], F16)
    is32_d = dram_in('is32', [C, C], F32)
    js32_d = dram_in('js32', [C, C], F32)

    y_d = nc.dram_tensor('y', [BL, N], F32, kind="ExternalOutput").ap()
    dbg = {}
    if DEBUG:
        dbg['h0'] = nc.dram_tensor('dbg_h0', [BL, C, N], F16, kind="ExternalOutput").ap()
        dbg['vt0'] = nc.dram_tensor('dbg_vt0', [128, 608], F16, kind="ExternalOutput").ap()
        dbg['vi0'] = nc.dram_tensor('dbg_vi0', [128, 512], F16, kind="ExternalOutput").ap()
        dbg['xs0'] = nc.dram_tensor('dbg_xs0', [64, 608], F16, kind="ExternalOutput").ap()
        dbg['pm0'] = nc.dram_tensor('dbg_pm0', [64, 576], F32, kind="ExternalOutput").ap()
        dbg['frs'] = nc.dram_tensor('dbg_frs', [C, NWP], F32, kind="ExternalOutput").ap()
        dbg['CT0'] = nc.dram_tensor('dbg_CT0', [128, C], F16, kind="ExternalOutput").ap()
        dbg['h1'] = nc.dram_tensor('dbg_h1', [BL, C, N], F16, kind="ExternalOutput").ap()

    mcols = [mode_col(u)[0] for u in range(288)]

    with tile.TileContext(nc) as tc, ExitStack() as ctx:
        # ------------- persistent pool -------------
        pp = ctx.enter_context(tc.tile_pool(name="persist", bufs=1))
        vt = [pp.tile([128, 32 * 640], F16, tag=f"vt{b}", name=f"vt{b}")
              for b in range(BL)]
        vinv = [pp.tile([128, 5 * 4096], F16, tag=f"vi{b}", name=f"vi{b}")
                for b in range(BL)]
        h = [pp.tile([C, N], F16, tag=f"h{b}", name=f"h{b}") for b in range(BL)]

        fc0w_t = pp.tile([2, C], F32R, tag="fc0w", name="fc0w_t")
        fc0b_t = pp.tile([C, 1], F32, tag="fc0b", name="fc0b_t")
        cwt_t = [pp.tile([C, C], F16, tag=f"cwt{l}", name=f"cwt{l}") for l in range(4)]
        cb_t = [pp.tile([C, 1], F32, tag=f"cb{l}", name=f"cb{l}") for l in range(4)]
        fc1w_t = pp.tile([C, 128], F16, tag="fc1w", name="fc1w_t")
        fc1b_t = pp.tile([128, 1], F32, tag="fc1b", name="fc1b_t")
        fc2w_t = pp.tile([128, 1], F16, tag="fc2w", name="fc2w_t")
        i64_t = pp.tile([C, C], F16, tag="i64", name="i64_t")
        i128_t = pp.tile([128, 128], F16, tag="i128", name="i128_t")
        is32_t = pp.tile([C, C], F32, tag="is32", name="is32_t")
        js32_t = pp.tile([C, C], F32, tag="js32", name="js32_t")

        nc.sync.dma_start(fc0w_t[:], fc0w_d[:])
        nc.sync.dma_start(fc0b_t[:], fc0b_d[:])
        for l in range(4):
            nc.sync.dma_start(cwt_t[l][:], cwt_d[l])
            nc.sync.dma_start(cb_t[l][:], cb_d[l])
        nc.sync.dma_start(fc1w_t[:], fc1w_d[:])
        nc.sync.dma_start(fc1b_t[:], fc1b_d[:])
        nc.sync.dma_start(fc2w_t[:], fc2w_d[:])
        nc.sync.dma_start(i64_t[:], i64_d[:])
        nc.sync.dma_start(i128_t[:], i128_d[:])
        nc.sync.dma_start(is32_t[:], is32_d[:])
        nc.sync.dma_start(js32_t[:], js32_d[:])

        # ------------- V build + fc0 -------------
        with tc.tile_pool(name="vbuild", bufs=1) as vb, \
             tc.tile_pool(name="vbps", bufs=1, space="PSUM") as vbps:
            selT_t = vb.tile([50, 608], F32R, tag="selT", name="selT_t")
            nc.sync.dma_start(selT_t[:], selT_d[:])

            # zero the 16-col pads of the [cos 304|z16|sin 304|z16] kt-blocks
            for b in range(BL):
                nc.vector.memset(
                    _cap(vt[b], 0, 128, [[320, 64], [1, 16]], 304), 0.0)

            cp_eng = 0
            for b in range(BL):
                for c8 in range(8):
                    cols = slice(512 * c8, 512 * (c8 + 1))
                    ckt = vb.tile([50, 512], F32R, tag="ck", bufs=2,
                                  name=f"ck{b}_{c8}")
                    nc.sync.dma_start(ckt[:], ck_d[b, :, cols])

                    ph0 = vbps.tile([C, 512], F32, tag="ph0", bufs=2,
                                    name=f"ph0_{b}_{c8}")
                    nc.tensor.matmul(ph0[:], fc0w_t[:], ckt[0:2, :],
                                     start=True, stop=True)
                    nc.scalar.activation(h[b][:, cols], ph0[:], AF.Identity,
                                         bias=fc0b_t[:, :])

                    for s in range(4):
                        kt = 4 * c8 + s
                        pv = vbps.tile([128, 640], F32, tag="pv", bufs=2,
                                       name=f"pv{b}_{kt}")
                        nc.tensor.matmul(pv[:, 0:512],
                                         ckt[:, 128 * s:128 * (s + 1)],
                                         selT_t[:, 0:512], start=True, stop=True)
                        nc.tensor.matmul(pv[:, 512:608],
                                         ckt[:, 128 * s:128 * (s + 1)],
                                         selT_t[:, 512:608], start=True, stop=True)
                        # pi/2 cos-shift comes in via the const ck row
                        nc.vector.add_range_wrap(pv[:, 0:608], pv[:, 0:608],
                                                 shift=0.0, bound=PI,
                                                 period=2 * PI)
                        nc.scalar.activation(
                            _cap(vt[b], 0, 128, [[320, 2], [1, 304]], 640 * kt),
                            pv[:, 0:608], AF.Sin)
                        tp = vbps.tile([128, 640], F16, tag="tp", bufs=2,
                                       name=f"tp{b}_{kt}")
                        # packed-640 rows: [cos 0:304 | z16 | sin 0:304 | z16]
                        for t in range(5):
                            nc.tensor.matmul(
                                tp[:, 128 * t:128 * t + 128],
                                vt[b][:, 640 * kt + 128 * t:
                                       640 * kt + 128 * (t + 1)],
                                i128_t[:], start=True, stop=True,
                                is_transpose=True)
                        dst = _cap(vinv[b], 0, 128, [[4096, 5], [1, 128]],
                                   128 * kt)
                        if cp_eng == 2:
                            nc.scalar.activation(dst, tp[:, :], AF.Copy)
                        else:
                            nc.vector.tensor_copy(dst, tp[:, :])
                        cp_eng = (cp_eng + 1) % 3

        if DEBUG:
            for b in range(BL):
                nc.sync.dma_start(dbg['h0'][b], h[b][:])
            nc.sync.dma_start(dbg['vt0'][:], vt[0][:, 0:608])
            nc.sync.dma_start(dbg['vi0'][:], vinv[0][:, 0:512])

        # ------------- layers -------------
        with tc.tile_pool(name="work", bufs=1) as wk, \
             tc.tile_pool(name="wkps", bufs=1, space="PSUM") as wkps:

            CT = [[wk.tile([128, C], F16, tag=f"CT{b}_{t}",
                           name=f"CT{b}_{t}") for t in range(5)]
                  for b in range(BL)]

            for l in range(4):
                last = (l == 3)
                # ---- slab prefetch for the mode mix ----
                slabs = []
                for c4 in range(4):
                    sE = wk.tile([64, 2304], F16, tag="sE", bufs=2,
                                 name=f"sE{l}_{c4}")
                    sO = wk.tile([64, 2304], F16, tag="sO", bufs=2,
                                 name=f"sO{l}_{c4}")
                    nc.sync.dma_start(sE[:], mmw_d[l, 0, :,
                                                   2304 * c4:2304 * (c4 + 1)])
                    nc.sync.dma_start(sO[:], mmw_d[l, 1, :,
                                                   2304 * c4:2304 * (c4 + 1)])
                    slabs.append((sE, sO))

                # ---- forward NUDFT (transpose one kt ahead) ----
                px = wkps.tile([128, NWP], F32, tag="px", bufs=1, name=f"px{l}")
                pt = wkps.tile([128, 128], F16, tag="pt", bufs=1, name=f"pt{l}")
                hTts = {}
                for kt in range(33):
                    if kt < 32:
                        reg = 64 * (kt % 2)
                        for b in range(BL):
                            nc.tensor.matmul(
                                pt[:, reg + 32 * b:reg + 32 * (b + 1)],
                                h[b][:, 128 * kt:128 * (kt + 1)],
                                i64_t[:], start=True, stop=True,
                                is_transpose=True)
                        hTt = wk.tile([128, 2 * C], F16, tag="hT", bufs=3,
                                      name=f"hT{l}_{kt}")
                        nc.vector.tensor_copy(hTt[:], pt[:, reg:reg + 64])
                        hTts[kt] = hTt
                    if kt >= 1:
                        k0 = kt - 1
                        hTt0 = hTts.pop(k0)
                        for g in range(4):
                            b, ri = g // 2, g % 2
                            rhs = vt[b][:, 640 * k0 + 320 * ri:
                                        640 * k0 + 320 * ri + 304]
                            nc.tensor.matmul(px[32 * g:32 * (g + 1), :],
                                             hTt0[:, 32 * b:32 * (b + 1)], rhs,
                                             start=(k0 == 0), stop=(k0 == 31),
                                             tile_position=(0, 32 * g))

                # ---- x_ft slab: xs2[:, 2w+b] = px[64b:64b+64, w] ----
                xs2 = wk.tile([64, 2 * NWP], F16, tag="xs2", bufs=1,
                              name=f"xs2_{l}")
                for b in range(BL):
                    nc.vector.tensor_copy(
                        _cap(xs2, 0, 64, [[2, NWP]], b),
                        px[64 * b:64 * (b + 1), :])

                if DEBUG and l == 0:
                    nc.sync.dma_start(dbg['xs0'][:], xs2[:])

                # ---- mode mix: 288 compact [64,64] aug matmuls ----
                pm = wkps.tile([64, 576], F32, tag="pm", bufs=1, name=f"pm{l}")
                for c4 in range(4):
                    sE, sO = slabs[c4]
                    for rr in range(36):
                        r = 36 * c4 + rr
                        for par in range(2):
                            u = 2 * r + par
                            mc = mcols[u]
                            st = sE if par == 0 else sO
                            nc.tensor.matmul(pm[:, 2 * u:2 * u + 2],
                                             st[:, 64 * rr:64 * (rr + 1)],
                                             xs2[:, 2 * mc:2 * mc + 2],
                                             start=True, stop=True)

                if DEBUG and l == 0:
                    nc.sync.dma_start(dbg['pm0'][:], pm[:])

                # ---- coefficient extraction ----
                frs = [wk.tile([C, NWP], F32, tag=f"frs{b}", name=f"frs{l}_{b}")
                       for b in range(BL)]
                fis = [wk.tile([C, NWP], F32, tag=f"fis{b}", name=f"fis{l}_{b}")
                       for b in range(BL)]
                frx = [wk.tile([C, NWP], F32, tag=f"frx{b}", name=f"frx{l}_{b}")
                       for b in range(BL)]
                fix = [wk.tile([C, NWP], F32, tag=f"fix{b}", name=f"fix{l}_{b}")
                       for b in range(BL)]
                for b in range(BL):
                    # unwritten cols only: frs/fis 288:304, frx/fix 0:24+299:304
                    nc.gpsimd.memset(frs[b][:, 288:NWP], 0.0)
                    nc.gpsimd.memset(fis[b][:, 288:NWP], 0.0)
                    nc.gpsimd.memset(frx[b][:], 0.0)
                    nc.gpsimd.memset(fix[b][:], 0.0)
                    nc.vector.tensor_copy(frs[b][:, 0:288],
                                          _cap(pm, 0, 32, [[2, 288]], b))
                    nc.scalar.activation(fis[b][:, 0:288],
                                         _cap(pm, 32, 32, [[2, 288]], b),
                                         AF.Copy)
                    # partner-coefficient slabs: frx on Act, fix on DVE
                    for (dst, src, eng) in ((frx[b], frs[b], 'act'),
                                            (fix[b], fis[b], 'dve')):
                        cp = (nc.scalar if eng == 'act' else nc.vector)
                        def _cpy(o, i, cp=cp, eng=eng):
                            if eng == 'act':
                                cp.activation(o, i, AF.Copy)
                            else:
                                cp.tensor_copy(o, i)
                        d3 = dst[:, 0:288].rearrange("p (j i) -> p j i", i=24)
                        s3 = src[:, 0:288].rearrange("p (j i) -> p j i", i=24)
                        _cpy(d3[:, 1:12, 1:12], s3[:, 1:12, 0:11])
                        _cpy(d3[:, 1:12, 13:24], s3[:, 1:12, 12:23])
                        _cpy(d3[:, 1:12, 0:1], s3[:, 1:12, 23:24])
                        _cpy(dst[:, 288:299],
                             s3[:, 11:0:-1, 11:12].rearrange("p j i -> p (j i)"))
                    nc.vector.tensor_scalar(fix[b][:, 288:299], fix[b][:, 288:299],
                                            -1.0, None, op0=ALU.mult)

                if DEBUG and l == 0:
                    nc.sync.dma_start(dbg['frs'][:], frs[0][:])

                # ---- packed coefficient slabs CT[b][t] ----
                # t-tile sources: (kind, col0, width, ctrow0)
                tspec = [[(0, 0, 128, 0)], [(0, 128, 128, 0)],
                         [(0, 256, 48, 0), (1, 0, 64, 64)],
                         [(1, 64, 128, 0)], [(1, 192, 112, 0)]]
                ct_eng = 0
                for b in range(BL):
                    if l == 0:
                        nc.gpsimd.memset(CT[b][2][32:64, :], 0.0)
                        nc.gpsimd.memset(CT[b][4][96:128, :], 0.0)
                    for t in range(5):
                        for (kind, c0, wdt, r0) in tspec[t]:
                            sd = frs[b] if kind == 0 else fis[b]
                            sf = frx[b] if kind == 0 else fix[b]
                            pc = wkps.tile([128, C], F32, tag="pc", bufs=2,
                                           name=f"pc{l}_{b}_{t}_{r0}")
                            nc.tensor.matmul(pc[0:wdt, :], sd[:, c0:c0 + wdt],
                                             is32_t[:], start=True, stop=False,
                                             is_transpose=True)
                            nc.tensor.matmul(pc[0:wdt, :], sf[:, c0:c0 + wdt],
                                             js32_t[:], start=False, stop=True,
                                             is_transpose=True)
                            dstap = CT[b][t][r0:r0 + wdt, :]
                            if ct_eng == 1:
                                nc.scalar.activation(dstap, pc[0:wdt, :],
                                                     AF.Copy,
                                                     scale=1.0 / 2048.0)
                            else:
                                nc.vector.tensor_scalar(dstap, pc[0:wdt, :],
                                                        1.0 / 2048.0, None,
                                                        op0=ALU.mult)
                            ct_eng = (ct_eng + 1) % 2

                if DEBUG and l == 0:
                    nc.sync.dma_start(dbg['CT0'][:], CT[0][0][:])

                # ---- inverse NUDFT + conv + activation ----
                for c8 in range(8):
                    cols = slice(512 * c8, 512 * (c8 + 1))
                    pi_ = wkps.tile([64, 512], F32, tag="pinv", bufs=2,
                                    name=f"pinv{l}_{c8}")
                    for b in range(BL):
                        sl = pi_[32 * b:32 * (b + 1), :]
                        tp_ = (0, 32 * b)
                        for t in range(5):
                            nc.tensor.matmul(
                                sl, CT[b][t][:],
                                vinv[b][:, 4096 * t + 512 * c8:
                                        4096 * t + 512 * (c8 + 1)],
                                start=(t == 0), stop=False, tile_position=tp_)
                        nc.tensor.matmul(sl, cwt_t[l][:], h[b][:, cols],
                                         start=False, stop=True,
                                         tile_position=tp_)
                    for b in range(BL):
                        nc.scalar.activation(
                            h[b][:, cols], pi_[32 * b:32 * (b + 1), :],
                            AF.Identity if last else AF.Gelu,
                            bias=cb_t[l][:, :])

        if DEBUG:
            for b in range(BL):
                nc.sync.dma_start(dbg['h1'][b], h[b][:])
        # ---- head: fc1 + gelu + fc2 ----
        with tc.tile_pool(name="head", bufs=1) as hd, \
             tc.tile_pool(name="hdps", bufs=1, space="PSUM") as hdps:
            for b in range(BL):
                for c4 in range(4):
                    cols = slice(1024 * c4, 1024 * (c4 + 1))
                    pg = hdps.tile([128, 1024], F32, tag="pg", bufs=2,
                                   name=f"pg{b}_{c4}")
                    for hh in range(2):
                        nc.tensor.matmul(pg[:, 512 * hh:512 * (hh + 1)],
                                         fc1w_t[:],
                                         h[b][:, 1024 * c4 + 512 * hh:
                                              1024 * c4 + 512 * (hh + 1)],
                                         start=True, stop=True)
                    g = hd.tile([128, 1024], F16, tag="g", bufs=2, name=f"g{b}_{c4}")
                    nc.scalar.activation(g[:], pg[:], AF.Gelu, bias=fc1b_t[:, :])
                    py = hdps.tile([1, 1024], F32, tag="py", bufs=2,
                                   name=f"py{b}_{c4}")
                    for hh in range(2):
                        nc.tensor.matmul(py[:, 512 * hh:512 * (hh + 1)],
                                         fc2w_t[:],
                                         g[:, 512 * hh:512 * (hh + 1)],
                                         start=True, stop=True)
                    ys = hd.tile([1, 1024], F32, tag="ys", bufs=2, name=f"ys{b}_{c4}")
                    nc.vector.tensor_copy(ys[:], py[:])
                    nc.sync.dma_start(y_d[b:b + 1, cols], ys[:])

    nc.compile()
    return nc


# --------------------------------------------------------------------------
# host marshaling
# --------------------------------------------------------------------------
def _marshal(pos, fc0_w, fc0_b, sw1r, sw1i, sw2r, sw2i, cw, cb,
             fc1_w, fc1_b, fc2_w, fc2_b):
    xp = (pos[:, :, 0] - pos[:, :, 0].min()).astype(np.float64)
    yp = (pos[:, :, 1] - pos[:, :, 1].min()).astype(np.float64)
    sx = np.float64(np.float32(6.28) / np.float32(xp.max()))
    sy = np.float64(np.float32(6.28) / np.float32(yp.max()))
    kx = np.concatenate([np.arange(MODES), np.arange(-MODES, 0)]).astype(np.float64)
    ky = np.concatenate([np.arange(MODES), np.arange(-(MODES - 1), 0)]).astype(np.float64)

    def wrap(v):
        return v - 2 * np.pi * np.round(v / (2 * np.pi))

    ck = np.zeros((B, 50, N), np.float32)
    ck[:, 0, :] = xp.astype(np.float32)
    ck[:, 1, :] = yp.astype(np.float32)
    for i in range(24):
        ck[:, 2 + i, :] = wrap(kx[i] * sx * xp).astype(np.float32)
    for j in range(23):
        ck[:, 26 + j, :] = wrap(ky[j] * sy * yp).astype(np.float32)
    ck[:, 49, :] = np.float32(np.pi / 2)

    worder = _w_rows()
    # selT [50, 608]: cols 0:304 = +phase+pi/2 (cos), 304:608 = -phase (-sin)
    selT = np.zeros((50, 608), np.float32)
    for w, m in enumerate(worder):
        i, j = m % 24, m // 24
        selT[2 + i, w] = 1.0
        selT[26 + j, w] = 1.0
        selT[49, w] = 1.0
        selT[2 + i, 304 + w] = -1.0
        selT[26 + j, 304 + w] = -1.0

    # compact augmented mode-mix weights with conj baked in
    mmw2 = np.zeros((4, 2, 64, 9216), np.float16)
    for l in range(4):
        w1 = sw1r[l].astype(np.float64) + 1j * sw1i[l].astype(np.float64)
        w2 = sw2r[l].astype(np.float64) + 1j * sw2i[l].astype(np.float64)
        for u in range(288):
            a, s = u // 12, u % 12
            wm = w1[:, :, a, s] if a < 12 else w2[:, :, a - 12, s]
            wr = wm.real.astype(np.float16)
            wi = wm.imag.astype(np.float16)
            _, cj = mode_col(u)
            r, par = u // 2, u % 2
            blk = mmw2[l, par, :, 64 * r:64 * (r + 1)]
            blk[0:32, 0:32] = wr
            blk[0:32, 32:64] = wi
            if cj:
                blk[32:64, 0:32] = wi
                blk[32:64, 32:64] = -wr
            else:
                blk[32:64, 0:32] = -wi
                blk[32:64, 32:64] = wr

    cwt = np.ascontiguousarray(cw.transpose(0, 2, 1)).astype(np.float16)
    cbm = cb.reshape(4, C, 1).astype(np.float32)

    eye32 = np.eye(C, dtype=np.float32)
    args = dict(
        selT=selT, mmw2=mmw2,
        fc0w=fc0_w.astype(np.float32), fc0b=fc0_b.reshape(C, 1).astype(np.float32),
        cwt=cwt, cb=cbm,
        fc1w=fc1_w.astype(np.float16), fc1b=fc1_b.reshape(128, 1).astype(np.float32),
        fc2w=fc2_w.reshape(128, 1).astype(np.float16),
        i64=eye32.astype(np.float16),
        i128=np.eye(128, dtype=np.float16),
        is32=eye32,
        js32=eye32[::-1].copy(),
    )
    return ck, args


def kernel(**inputs):
    pos = np.asarray(inputs['pos'])
    ck, shared = _marshal(**{k: np.asarray(v) for k, v in inputs.items()})

    if 'nc' not in _CACHE:
        _CACHE['nc'] = _build_program()
    nc = _CACHE['nc']

    in_maps = []
    for core in range(NCORES):
        m = dict(shared)
        m['ck'] = ck[BL * core:BL * (core + 1)]
        in_maps.append(m)

    res = run_bass_kernel_spmd(nc, in_maps, list(range(NCORES)), trace=TRACE)
    _CACHE['last_results'] = res

    fc2_b = np.asarray(inputs['fc2_b']).astype(np.float32)
    out = np.zeros((B, N, 1), np.float32)
    for core in range(NCORES):
        out[BL * core:BL * (core + 1), :, 0] = res.results[core]['y']
    out += fc2_b.reshape(1, 1, 1)
    return out
